# revision 1
# baseline (speedup 1.0000x reference)
"""MLA attention kernel for Trainium2, 8 NeuronCores.

Sharding: core = (batch b in {0,1}) x (head-group hg in {0..3}, 4 heads each).
Each core computes the down-projections for its batch (replicated within the
4-core batch group), its 4 heads' up-projections + RoPE + causal attention +
the partial o_proj contribution of its heads. Host sums the 4 partial outputs
per batch.

All device matmuls are transpose-free: the host passes hidden[b]^T, and every
intermediate is produced in the layout its consumer needs:
  q_lat^T [768,S] -> q_nope^T/q_rope^T [d,S] (d-major)   (scores lhsT/rhs)
  c_kv^T [512,S]  -> k_nope^T/k_rope^T [d,S], V [S,dv]   (token-major V = PV lhsT)
  scores^T [k,q] -> exp -> probs^T (PV rhs), col-sum via ones-matmul,
  out^T [dv,q] normalized on PSUM-evict -> o_proj lhsT.
Rope dims are stored pair-packed: two heads per [128,S] tile (head A rows
0:64, head B rows 64:128); matmul APs slice at partition base 0/64.
Softmax skips fully-masked k/q tile pairs (causal) and uses no max-subtraction
(scores*scale is ~N(0,1); |s|<~8 so exp is safe in f32).
Phase order D2->U2->D1->U1->ATT gives stack-nestable SBUF pool lifetimes
(persistent outputs on the right side, transients on the left).
"""
import sys

sys.path.insert(0, "/opt/trn_rl_repo")

import numpy as np
import concourse.bass as bass
import concourse.bacc as bacc
import concourse.tile as tile
from concourse import mybir
from concourse.bass_utils import run_bass_kernel_spmd

FP = mybir.dt.float32
S = 2048
HID = 2048
H = 16
DN = 128
DR = 64
DV = 128
QL = 768
KVL = 512
ROPE_BASE = 10000.0
SCALE = (DN + DR) ** -0.5
NEG = -1e9
NCORES = 8
HPC = 4  # heads per core
P = 128
NB = S // 512  # 4 query/key column blocks of 512
KT = S // P  # 16 token tiles of 128
QLT = QL // P  # 6
KVT = KVL // P  # 4

_cache = {}


def _build(variant):
    """variant: 'causal' (on-chip mask + tile skipping), 'zeros' (no mask),
    'generic' (mask^T DMA'd from DRAM, all tiles)."""
    nc = bacc.Bacc()

    hidT = nc.dram_tensor("hidT", [HID, S], FP, kind="ExternalInput")
    w_qd = nc.dram_tensor("w_qd", [HID, QL], FP, kind="ExternalInput")
    w_kvd = nc.dram_tensor("w_kvd", [HID, KVL], FP, kind="ExternalInput")
    w_qu = nc.dram_tensor("w_qu", [QL, HPC * DN], FP, kind="ExternalInput")
    w_qr = nc.dram_tensor("w_qr", [QL, HPC * DR], FP, kind="ExternalInput")
    w_ku = nc.dram_tensor("w_ku", [KVL, HPC * DN], FP, kind="ExternalInput")
    w_kr = nc.dram_tensor("w_kr", [KVL, HPC * DR], FP, kind="ExternalInput")
    w_vu = nc.dram_tensor("w_vu", [KVL, HPC * DV], FP, kind="ExternalInput")
    w_o = nc.dram_tensor("w_o", [HPC * DV, HID], FP, kind="ExternalInput")
    cs = nc.dram_tensor("cs", [128, 2 * S], FP, kind="ExternalInput")
    if variant == "generic":
        maskT = nc.dram_tensor("maskT", [S, S], FP, kind="ExternalInput")
    o_out = nc.dram_tensor("o", [S, HID], FP, kind="ExternalOutput")

    def down_proj(tc, w_dram, n_m, out_tiles, scale):
        # out[m][:, nb] (n_m tiles [128,S]) = w^T @ hidT, k-inner multi-psum
        with tc.tile_pool(name="wdp", bufs=1) as wp, \
             tc.tile_pool(name="rhs", bufs=4) as rp, \
             tc.tile_pool(name="psd", bufs=1, space="PSUM") as pp:
            wt = [wp.tile([P, n_m * P], FP, name=f"wd{k}", tag=f"wd{k}")
                  for k in range(KT)]
            for k in range(KT):
                nc.sync.dma_start(out=wt[k][:], in_=w_dram[k * P:(k + 1) * P, :])
            for n in range(NB):
                pss = [pp.tile([P, 512], FP, name=f"psd{m}", tag=f"psd{m}")
                       for m in range(n_m)]
                for k in range(KT):
                    r = rp.tile([P, 512], FP, name="rhs", tag="rhs")
                    nc.sync.dma_start(
                        out=r[:], in_=hidT[k * P:(k + 1) * P, n * 512:(n + 1) * 512])
                    for m in range(n_m):
                        nc.tensor.matmul(pss[m][:], wt[k][:, m * P:(m + 1) * P], r[:],
                                         start=(k == 0), stop=(k == KT - 1))
                for m in range(n_m):
                    if scale is not None:
                        nc.scalar.activation(out_tiles[m][:, n * 512:(n + 1) * 512],
                                             pss[m][:],
                                             mybir.ActivationFunctionType.Copy,
                                             scale=float(scale))
                    elif m % 2 == 0:
                        nc.scalar.copy(out_tiles[m][:, n * 512:(n + 1) * 512], pss[m][:])
                    else:
                        nc.vector.tensor_copy(out_tiles[m][:, n * 512:(n + 1) * 512],
                                              pss[m][:])

    def rope_block(tc, x, csp, tp, n):
        # in-place rope on x[:, n*512:(n+1)*512] of a pair-packed [128,S] tile
        cob = csp.tile([P, 512], FP, name="cob", tag="cob")
        nc.sync.dma_start(out=cob[:], in_=cs[:, n * 512:(n + 1) * 512])
        snb = csp.tile([P, 512], FP, name="snb", tag="snb")
        nc.sync.dma_start(out=snb[:], in_=cs[:, S + n * 512:S + (n + 1) * 512])
        xb = x[:, n * 512:(n + 1) * 512]
        t2 = tp.tile([P, 512], FP, name="t2", tag="t2")
        for q in range(4):
            src = (q // 2) * 64 + (32 if q % 2 == 0 else 0)
            # partition-shifted copy (1-input; shifted 2-input TT is rejected
            # by the compiler: base partitions must match)
            nc.vector.tensor_copy(t2[q * 32:(q + 1) * 32], xb[src:src + 32])
        nc.vector.tensor_tensor(t2[:], t2[:], snb[:], mybir.AluOpType.mult)
        nc.vector.tensor_tensor(xb, xb, cob[:], mybir.AluOpType.mult)
        nc.vector.tensor_tensor(xb, xb, t2[:], mybir.AluOpType.add)

    def up_proj(tc, wp, w_sb, kt, rhs_tiles, out_tile, h_cols, n, ev):
        # out_tile[:, nb] = w_sb[:, h_cols]^T @ rhs, contraction kt tiles
        ps = upp[0].tile([P, 512], FP, name="psu", tag="psu")
        for k in range(kt):
            nc.tensor.matmul(ps[:], w_sb[k][:, h_cols], rhs_tiles[k][:, n * 512:(n + 1) * 512],
                             start=(k == 0), stop=(k == kt - 1))
        if ev == 0:
            nc.scalar.copy(out_tile[:, n * 512:(n + 1) * 512], ps[:])
        else:
            nc.vector.tensor_copy(out_tile[:, n * 512:(n + 1) * 512], ps[:])

    with tile.TileContext(nc) as tc:
        with tc.tile_pool(name="kvout", bufs=1, side="right") as kvp:
            k_nope = [kvp.tile([P, S], FP, name=f"kn{h}", tag=f"kn{h}")
                      for h in range(HPC)]
            k_rope = [kvp.tile([P, S], FP, name=f"kr{p}", tag=f"kr{p}")
                      for p in range(HPC // 2)]
            v4 = [kvp.tile([P, HPC * DV], FP, name=f"v{t}", tag=f"v{t}")
                  for t in range(KT)]

            # ---- D2: c_kv^T ----
            with tc.tile_pool(name="ckvp", bufs=1) as cp_:
                c_kv = [cp_.tile([P, S], FP, name=f"ckv{m}", tag=f"ckv{m}")
                        for m in range(KVT)]
                down_proj(tc, w_kvd, KVT, c_kv, None)

                # ---- U2: k-ups + rope-k + V4 ----
                with tc.tile_pool(name="wku", bufs=1) as wp, \
                     tc.tile_pool(name="csp2", bufs=4) as csp, \
                     tc.tile_pool(name="tp2", bufs=2) as tp, \
                     tc.tile_pool(name="psu2", bufs=3, space="PSUM") as pu:
                    upp = [pu]
                    ku = [wp.tile([P, HPC * DN], FP, name=f"wku{k}", tag=f"wku{k}")
                          for k in range(KVT)]
                    kr = [wp.tile([P, HPC * DR], FP, name=f"wkr{k}", tag=f"wkr{k}")
                          for k in range(KVT)]
                    vu = [wp.tile([P, HPC * DV], FP, name=f"wvu{k}", tag=f"wvu{k}")
                          for k in range(KVT)]
                    for k in range(KVT):
                        nc.sync.dma_start(out=ku[k][:], in_=w_ku[k * P:(k + 1) * P, :])
                        nc.sync.dma_start(out=kr[k][:], in_=w_kr[k * P:(k + 1) * P, :])
                        nc.sync.dma_start(out=vu[k][:], in_=w_vu[k * P:(k + 1) * P, :])
                    for n in range(NB):
                        for h in range(HPC):
                            up_proj(tc, pu, ku, KVT, c_kv, k_nope[h],
                                    slice(h * DN, (h + 1) * DN), n, h % 2)
                        for p in range(HPC // 2):
                            up_proj(tc, pu, kr, KVT, c_kv, k_rope[p],
                                    slice(p * 2 * DR, (p + 1) * 2 * DR), n, p % 2)
                            rope_block(tc, k_rope[p], csp, tp, n)
                    for t in range(KT):
                        ps = pu.tile([P, HPC * DV], FP, name="psv", tag="psv")
                        for k in range(KVT):
                            nc.tensor.matmul(ps[:], c_kv[k][:, t * P:(t + 1) * P],
                                             vu[k][:], start=(k == 0),
                                             stop=(k == KVT - 1))
                        if t % 2 == 0:
                            nc.scalar.copy(v4[t][:], ps[:])
                        else:
                            nc.vector.tensor_copy(v4[t][:], ps[:])

            # ---- D1: q_lat^T (scaled) ----  (before qout opens; left stack)
            qlp = tc.alloc_tile_pool(name="qlatp", bufs=1)
            q_lat = [qlp.tile([P, S], FP, name=f"ql{m}", tag=f"ql{m}")
                     for m in range(QLT)]
            down_proj(tc, w_qd, QLT, q_lat, SCALE)

            with tc.tile_pool(name="qout", bufs=1, side="right") as qp:
                q_nope = [qp.tile([P, S], FP, name=f"qn{h}", tag=f"qn{h}")
                          for h in range(HPC)]
                q_rope = [qp.tile([P, S], FP, name=f"qr{p}", tag=f"qr{p}")
                          for p in range(HPC // 2)]

                # ---- U1: q-ups + rope-q ----
                with tc.tile_pool(name="wqup", bufs=1) as wp, \
                     tc.tile_pool(name="csp1", bufs=2) as csp, \
                     tc.tile_pool(name="tp1", bufs=1) as tp, \
                     tc.tile_pool(name="psu1", bufs=4, space="PSUM") as pu:
                    upp = [pu]
                    wu = [wp.tile([P, HPC * DN], FP, name=f"wqu{k}", tag=f"wqu{k}")
                          for k in range(QLT)]
                    wr = [wp.tile([P, HPC * DR], FP, name=f"wqr{k}", tag=f"wqr{k}")
                          for k in range(QLT)]
                    for k in range(QLT):
                        nc.sync.dma_start(out=wu[k][:], in_=w_qu[k * P:(k + 1) * P, :])
                        nc.sync.dma_start(out=wr[k][:], in_=w_qr[k * P:(k + 1) * P, :])
                    for n in range(NB):
                        for h in range(HPC):
                            up_proj(tc, pu, wu, QLT, q_lat, q_nope[h],
                                    slice(h * DN, (h + 1) * DN), n, h % 2)
                        for p in range(HPC // 2):
                            up_proj(tc, pu, wr, QLT, q_lat, q_rope[p],
                                    slice(p * 2 * DR, (p + 1) * 2 * DR), n, p % 2)
                            rope_block(tc, q_rope[p], csp, tp, n)
                qlp.release()

                # ---- ATT + o_proj per q-block ----
                with tc.tile_pool(name="att_c", bufs=1) as cp, \
                     tc.tile_pool(name="probs", bufs=4) as prp, \
                     tc.tile_pool(name="attn", bufs=5) as atp, \
                     tc.tile_pool(name="osb", bufs=2) as osp, \
                     tc.tile_pool(name="rdn", bufs=2) as rdp, \
                     tc.tile_pool(name="rbp", bufs=2) as rbp, \
                     tc.tile_pool(name="ps_s", bufs=2, space="PSUM") as ps_s, \
                     tc.tile_pool(name="ps_pv", bufs=2, space="PSUM") as ps_pv, \
                     tc.tile_pool(name="ps_den", bufs=1, space="PSUM") as ps_den, \
                     tc.tile_pool(name="ps_bc", bufs=1, space="PSUM") as ps_bc, \
                     tc.tile_pool(name="ps_o", bufs=2, space="PSUM") as ps_o:
                    wo = [cp.tile([P, HID], FP, name=f"wo{k}", tag=f"wo{k}")
                          for k in range(HPC)]
                    for k in range(HPC):
                        nc.sync.dma_start(out=wo[k][:], in_=w_o[k * P:(k + 1) * P, :])
                    ones_k = cp.tile([P, 1], FP, name="ones_k", tag="ones_k")
                    nc.vector.memset(ones_k[:], 1.0)
                    ones_m = cp.tile([1, P], FP, name="ones_m", tag="ones_m")
                    nc.vector.memset(ones_m[:], 1.0)
                    if variant == "causal":
                        msk = cp.tile([P, 896], FP, name="msk", tag="msk")
                        nc.gpsimd.memset(msk[:], 0.0)
                        nc.gpsimd.affine_select(
                            out=msk[:], in_=msk[:],
                            compare_op=mybir.AluOpType.is_ge,
                            fill=NEG, base=-384,
                            pattern=[[1, 896]], channel_multiplier=-1)
                    if variant == "generic":
                        mrp = tc.alloc_tile_pool(name="mrhs", bufs=18)

                    for j in range(NB):
                        nkt = 4 * (j + 1) if variant == "causal" else KT
                        mts = []
                        if variant == "generic":
                            for ki in range(KT):
                                mt = mrp.tile([P, 512], FP, name="mrhs", tag="mrhs")
                                nc.sync.dma_start(
                                    out=mt[:],
                                    in_=maskT[ki * P:(ki + 1) * P,
                                              j * 512:(j + 1) * 512])
                                mts.append(mt)
                        attn_sb = []
                        for h in range(HPC):
                            pp_, hh = h // 2, (h % 2) * DR
                            pv = ps_pv.tile([P, 512], FP)
                            den = ps_den.tile([1, 512], FP)
                            for ki in range(nkt):
                                ss = ps_s.tile([P, 512], FP)
                                nc.tensor.matmul(
                                    ss[:], k_nope[h][:, ki * P:(ki + 1) * P],
                                    q_nope[h][:, j * 512:(j + 1) * 512],
                                    start=True, stop=False)
                                nc.tensor.matmul(
                                    ss[:],
                                    k_rope[pp_][hh:hh + DR, ki * P:(ki + 1) * P],
                                    q_rope[pp_][hh:hh + DR, j * 512:(j + 1) * 512],
                                    start=False, stop=True)
                                pr = prp.tile([P, 512], FP, name="pr", tag="pr")
                                off = 128 * ki - 512 * j
                                if variant == "causal" and off >= 0:
                                    c0 = 384 - off
                                    nc.vector.tensor_tensor(
                                        pr[:], ss[:], msk[:, c0:c0 + 512],
                                        mybir.AluOpType.add)
                                    nc.scalar.activation(
                                        pr[:], pr[:],
                                        mybir.ActivationFunctionType.Exp)
                                elif variant == "generic":
                                    nc.vector.tensor_tensor(
                                        pr[:], ss[:], mts[ki][:],
                                        mybir.AluOpType.add)
                                    nc.scalar.activation(
                                        pr[:], pr[:],
                                        mybir.ActivationFunctionType.Exp)
                                else:
                                    nc.scalar.activation(
                                        pr[:], ss[:],
                                        mybir.ActivationFunctionType.Exp)
                                nc.tensor.matmul(pv[:],
                                                 v4[ki][:, h * DV:(h + 1) * DV],
                                                 pr[:], start=(ki == 0),
                                                 stop=(ki == nkt - 1))
                                nc.tensor.matmul(den[:], ones_k[:], pr[:],
                                                 start=(ki == 0),
                                                 stop=(ki == nkt - 1))
                            rden = rdp.tile([1, 512], FP, name="rden", tag="rden")
                            nc.vector.reciprocal(rden[:], den[:])
                            bc = ps_bc.tile([P, 512], FP)
                            nc.tensor.matmul(bc[:], ones_m[:], rden[:],
                                             start=True, stop=True)
                            rb = rbp.tile([P, 512], FP, name="rb", tag="rb")
                            nc.scalar.copy(rb[:], bc[:])
                            at = atp.tile([P, 512], FP, name="at", tag="at")
                            nc.vector.tensor_tensor(at[:], pv[:], rb[:],
                                                    mybir.AluOpType.mult)
                            attn_sb.append(at)
                        for t in range(4):
                            ob = osp.tile([P, HID], FP, name="ob", tag="ob")
                            for nn in range(NB):
                                po = ps_o.tile([P, 512], FP)
                                for kk in range(HPC):
                                    nc.tensor.matmul(
                                        po[:], attn_sb[kk][:, t * P:(t + 1) * P],
                                        wo[kk][:, nn * 512:(nn + 1) * 512],
                                        start=(kk == 0), stop=(kk == HPC - 1))
                                if nn % 2 == 0:
                                    nc.scalar.copy(ob[:, nn * 512:(nn + 1) * 512],
                                                   po[:])
                                else:
                                    nc.vector.tensor_copy(
                                        ob[:, nn * 512:(nn + 1) * 512], po[:])
                            nc.sync.dma_start(
                                out=o_out[(j * 4 + t) * P:(j * 4 + t + 1) * P, :],
                                in_=ob[:])
                    if variant == "generic":
                        mrp.release()

    nc.compile()
    return nc


def _get(variant):
    if variant not in _cache:
        _cache[variant] = _build(variant)
    return _cache[variant]


def _host_prep(inputs):
    hs = np.ascontiguousarray(inputs["hidden_states"], dtype=np.float32)
    mask = np.asarray(inputs["attention_mask"], dtype=np.float32)
    pos = np.asarray(inputs["position_ids"])
    B = hs.shape[0]

    causal = np.where(np.tril(np.ones((S, S), dtype=bool)), np.float32(0.0),
                      np.float32(NEG))
    variant = "causal"
    for b in range(B):
        if not np.array_equal(mask[b, 0], causal):
            variant = "zeros" if not mask.any() else "generic"
            break

    inv_freq = (1.0 / (ROPE_BASE ** (np.arange(0, DR, 2, dtype=np.float32) / DR)))
    css = []
    for b in range(B):
        t = pos[b].astype(np.float32)
        freqs = t[:, None] * inv_freq[None, :]  # [S, 32]
        cf = np.cos(freqs).T  # [32, S]
        sf = np.sin(freqs).T
        cs = np.empty((128, 2 * S), dtype=np.float32)
        for q in range(4):
            cs[q * 32:(q + 1) * 32, :S] = cf
            cs[q * 32:(q + 1) * 32, S:] = sf if q % 2 else -sf
        css.append(np.ascontiguousarray(cs))
    return hs, mask, css, variant


def kernel(**inputs):
    hs, mask, css, variant = _host_prep(inputs)
    nc = _get(variant)

    w_qd = np.ascontiguousarray(inputs["W_q_down"], dtype=np.float32)
    w_kvd = np.ascontiguousarray(inputs["W_kv_down"], dtype=np.float32)
    W_qu = np.asarray(inputs["W_q_up"], dtype=np.float32)
    W_qr = np.asarray(inputs["W_q_rope"], dtype=np.float32)
    W_ku = np.asarray(inputs["W_k_up"], dtype=np.float32)
    W_kr = np.asarray(inputs["W_k_rope"], dtype=np.float32)
    W_vu = np.asarray(inputs["W_v_up"], dtype=np.float32)
    W_o = np.asarray(inputs["W_o"], dtype=np.float32)

    hidT = [np.ascontiguousarray(hs[b].T) for b in range(2)]
    maskT = [np.ascontiguousarray(mask[b, 0].T) for b in range(2)] \
        if variant == "generic" else None

    in_maps = []
    for core in range(NCORES):
        b, hg = divmod(core, NCORES // 2)
        m = {
            "hidT": hidT[b],
            "w_qd": w_qd,
            "w_kvd": w_kvd,
            "w_qu": np.ascontiguousarray(W_qu[:, hg * HPC * DN:(hg + 1) * HPC * DN]),
            "w_qr": np.ascontiguousarray(W_qr[:, hg * HPC * DR:(hg + 1) * HPC * DR]),
            "w_ku": np.ascontiguousarray(W_ku[:, hg * HPC * DN:(hg + 1) * HPC * DN]),
            "w_kr": np.ascontiguousarray(W_kr[:, hg * HPC * DR:(hg + 1) * HPC * DR]),
            "w_vu": np.ascontiguousarray(W_vu[:, hg * HPC * DV:(hg + 1) * HPC * DV]),
            "w_o": np.ascontiguousarray(W_o[hg * HPC * DV:(hg + 1) * HPC * DV, :]),
            "cs": css[b],
        }
        if maskT is not None:
            m["maskT"] = maskT[b]
        in_maps.append(m)

    res = run_bass_kernel_spmd(nc, in_maps, core_ids=list(range(NCORES)))
    out = np.zeros((2, S, HID), dtype=np.float32)
    for core in range(NCORES):
        b = core // (NCORES // 2)
        out[b] += res.results[core]["o"]
    return out



# revision 5
# speedup vs baseline: 2.6451x; 2.6451x over previous
"""MLA attention kernel for Trainium2, 8 NeuronCores.

Sharding: core = (batch b in {0,1}) x (head-group hg in {0..3}, 4 heads each).
Each core computes the down-projections for its batch (replicated within the
4-core batch group), its 4 heads' up-projections + RoPE + causal attention +
the partial o_proj contribution of its heads. Host sums the 4 partial outputs
per batch.

All matmul operands are bf16 (1 PE cycle/row vs 4 for fp32); accumulation
stays fp32 in PSUM, and attention scores stay fp32 through mask-add + exp
(only probs are rounded to bf16). Host casts hidden^T / weights / rope table
to bf16; the o_proj output is produced and DMA'd in fp32.

All device matmuls are transpose-free: the host passes hidden[b]^T, and every
intermediate is produced in the layout its consumer needs:
  q_lat^T [768,S] -> q_nope^T/q_rope^T [d,S] (d-major)   (scores lhsT/rhs)
  c_kv^T [512,S]  -> k_nope^T/k_rope^T [d,S], V [S,dv]   (token-major V = PV lhsT)
  scores^T [k,q] -> exp -> probs^T (PV rhs), col-sum via ones-matmul,
  out^T [dv,q] normalized on PSUM-evict -> o_proj lhsT.
Rope dims are stored pair-packed: two heads per [128,S] tile (head A rows
0:64, head B rows 64:128); matmul APs slice at partition base 0/64.
Softmax skips fully-masked k/q tile pairs (causal) and uses no max-subtraction
(scores*scale is ~N(0,1); |s|<~8 so exp is safe in f32).
Phase order D2->U2->D1->U1->ATT gives stack-nestable SBUF pool lifetimes
(persistent outputs on the right side, transients on the left).
"""
import sys

sys.path.insert(0, "/opt/trn_rl_repo")

import numpy as np
import ml_dtypes
import concourse.bass as bass
import concourse.bacc as bacc
import concourse.tile as tile
from concourse import mybir
from concourse.bass_utils import run_bass_kernel_spmd

FP = mybir.dt.float32
BF = mybir.dt.bfloat16
NPBF = ml_dtypes.bfloat16
S = 2048
HID = 2048
H = 16
DN = 128
DR = 64
DV = 128
QL = 768
KVL = 512
ROPE_BASE = 10000.0
SCALE = (DN + DR) ** -0.5
NEG = -1e9
NCORES = 8
HPC = 4  # heads per core
P = 128
NB = S // 512  # 4 query/key column blocks of 512
KT = S // P  # 16 token tiles of 128
QLT = QL // P  # 6
KVT = KVL // P  # 4

_cache = {}


def _build(variant):
    """variant: 'causal' (on-chip mask + tile skipping), 'zeros' (no mask),
    'generic' (mask^T DMA'd from DRAM, all tiles)."""
    nc = bacc.Bacc()

    hidT = nc.dram_tensor("hidT", [HID, S], BF, kind="ExternalInput")
    w_qd = nc.dram_tensor("w_qd", [HID, QL], BF, kind="ExternalInput")
    w_kvd = nc.dram_tensor("w_kvd", [HID, KVL], BF, kind="ExternalInput")
    w_qu = nc.dram_tensor("w_qu", [QL, HPC * DN], BF, kind="ExternalInput")
    w_qr = nc.dram_tensor("w_qr", [QL, HPC * DR], BF, kind="ExternalInput")
    w_ku = nc.dram_tensor("w_ku", [KVL, HPC * DN], BF, kind="ExternalInput")
    w_kr = nc.dram_tensor("w_kr", [KVL, HPC * DR], BF, kind="ExternalInput")
    w_vu = nc.dram_tensor("w_vu", [KVL, HPC * DV], BF, kind="ExternalInput")
    w_o = nc.dram_tensor("w_o", [HPC * DV, HID], BF, kind="ExternalInput")
    cs = nc.dram_tensor("cs", [128, 2 * S], BF, kind="ExternalInput")
    if variant == "generic":
        maskT = nc.dram_tensor("maskT", [S, S], FP, kind="ExternalInput")
    o_out = nc.dram_tensor("o", [S, HID], FP, kind="ExternalOutput")

    def down_proj(tc, w_dram, n_m, out_tiles, scale):
        # out[m][:, nb] (n_m tiles [128,S]) = w^T @ hidT, k-inner multi-psum
        with tc.tile_pool(name="wdp", bufs=1) as wp, \
             tc.tile_pool(name="rhs", bufs=4) as rp, \
             tc.tile_pool(name="psd", bufs=1, space="PSUM") as pp:
            wt = [wp.tile([P, n_m * P], BF, name=f"wd{k}", tag=f"wd{k}")
                  for k in range(KT)]
            for k in range(KT):
                nc.sync.dma_start(out=wt[k][:], in_=w_dram[k * P:(k + 1) * P, :])
            for n in range(NB):
                pss = [pp.tile([P, 512], FP, name=f"psd{m}", tag=f"psd{m}")
                       for m in range(n_m)]
                for k in range(KT):
                    r = rp.tile([P, 512], BF, name="rhs", tag="rhs")
                    nc.sync.dma_start(
                        out=r[:], in_=hidT[k * P:(k + 1) * P, n * 512:(n + 1) * 512])
                    for m in range(n_m):
                        nc.tensor.matmul(pss[m][:], wt[k][:, m * P:(m + 1) * P], r[:],
                                         start=(k == 0), stop=(k == KT - 1))
                for m in range(n_m):
                    if scale is not None:
                        nc.scalar.activation(out_tiles[m][:, n * 512:(n + 1) * 512],
                                             pss[m][:],
                                             mybir.ActivationFunctionType.Copy,
                                             scale=float(scale))
                    elif m % 2 == 0:
                        nc.scalar.copy(out_tiles[m][:, n * 512:(n + 1) * 512], pss[m][:])
                    else:
                        nc.vector.tensor_copy(out_tiles[m][:, n * 512:(n + 1) * 512],
                                              pss[m][:])

    def rope_block(tc, x, csp, tp, n):
        # in-place rope on x[:, n*512:(n+1)*512] of a pair-packed [128,S] tile
        cob = csp.tile([P, 512], BF, name="cob", tag="cob")
        nc.sync.dma_start(out=cob[:], in_=cs[:, n * 512:(n + 1) * 512])
        snb = csp.tile([P, 512], BF, name="snb", tag="snb")
        nc.sync.dma_start(out=snb[:], in_=cs[:, S + n * 512:S + (n + 1) * 512])
        xb = x[:, n * 512:(n + 1) * 512]
        t2 = tp.tile([P, 512], BF, name="t2", tag="t2")
        for q in range(4):
            src = (q // 2) * 64 + (32 if q % 2 == 0 else 0)
            # partition-shifted copy (1-input; shifted 2-input TT is rejected
            # by the compiler: base partitions must match)
            nc.vector.tensor_copy(t2[q * 32:(q + 1) * 32], xb[src:src + 32])
        nc.vector.tensor_tensor(t2[:], t2[:], snb[:], mybir.AluOpType.mult)
        nc.vector.tensor_tensor(xb, xb, cob[:], mybir.AluOpType.mult)
        nc.vector.tensor_tensor(xb, xb, t2[:], mybir.AluOpType.add)

    def up_proj(tc, wp, w_sb, kt, rhs_tiles, out_tile, h_cols, n, ev):
        # out_tile[:, nb] = w_sb[:, h_cols]^T @ rhs, contraction kt tiles
        ps = upp[0].tile([P, 512], FP, name="psu", tag="psu")
        for k in range(kt):
            nc.tensor.matmul(ps[:], w_sb[k][:, h_cols], rhs_tiles[k][:, n * 512:(n + 1) * 512],
                             start=(k == 0), stop=(k == kt - 1))
        if ev == 0:
            nc.scalar.copy(out_tile[:, n * 512:(n + 1) * 512], ps[:])
        else:
            nc.vector.tensor_copy(out_tile[:, n * 512:(n + 1) * 512], ps[:])

    with nc.allow_low_precision(reason="bf16 kernel; 2e-2 rel-err budget"), \
         tile.TileContext(nc) as tc:
        with tc.tile_pool(name="kvout", bufs=1, side="right") as kvp:
            k_nope = [kvp.tile([P, S], BF, name=f"kn{h}", tag=f"kn{h}")
                      for h in range(HPC)]
            k_rope = [kvp.tile([P, S], BF, name=f"kr{p}", tag=f"kr{p}")
                      for p in range(HPC // 2)]
            v4 = [kvp.tile([P, HPC * DV], BF, name=f"v{t}", tag=f"v{t}")
                  for t in range(KT)]

            # ---- D2: c_kv^T ----
            with tc.tile_pool(name="ckvp", bufs=1) as cp_:
                c_kv = [cp_.tile([P, S], BF, name=f"ckv{m}", tag=f"ckv{m}")
                        for m in range(KVT)]
                down_proj(tc, w_kvd, KVT, c_kv, None)

                # ---- U2: k-ups + rope-k + V4 ----
                with tc.tile_pool(name="wku", bufs=1) as wp, \
                     tc.tile_pool(name="csp2", bufs=4) as csp, \
                     tc.tile_pool(name="tp2", bufs=2) as tp, \
                     tc.tile_pool(name="psu2", bufs=3, space="PSUM") as pu:
                    upp = [pu]
                    ku = [wp.tile([P, HPC * DN], BF, name=f"wku{k}", tag=f"wku{k}")
                          for k in range(KVT)]
                    kr = [wp.tile([P, HPC * DR], BF, name=f"wkr{k}", tag=f"wkr{k}")
                          for k in range(KVT)]
                    vu = [wp.tile([P, HPC * DV], BF, name=f"wvu{k}", tag=f"wvu{k}")
                          for k in range(KVT)]
                    for k in range(KVT):
                        nc.sync.dma_start(out=ku[k][:], in_=w_ku[k * P:(k + 1) * P, :])
                        nc.sync.dma_start(out=kr[k][:], in_=w_kr[k * P:(k + 1) * P, :])
                        nc.sync.dma_start(out=vu[k][:], in_=w_vu[k * P:(k + 1) * P, :])
                    for n in range(NB):
                        for h in range(HPC):
                            up_proj(tc, pu, ku, KVT, c_kv, k_nope[h],
                                    slice(h * DN, (h + 1) * DN), n, h % 2)
                        for p in range(HPC // 2):
                            up_proj(tc, pu, kr, KVT, c_kv, k_rope[p],
                                    slice(p * 2 * DR, (p + 1) * 2 * DR), n, p % 2)
                            rope_block(tc, k_rope[p], csp, tp, n)
                    for t in range(KT):
                        ps = pu.tile([P, HPC * DV], FP, name="psv", tag="psv")
                        for k in range(KVT):
                            nc.tensor.matmul(ps[:], c_kv[k][:, t * P:(t + 1) * P],
                                             vu[k][:], start=(k == 0),
                                             stop=(k == KVT - 1))
                        if t % 2 == 0:
                            nc.scalar.copy(v4[t][:], ps[:])
                        else:
                            nc.vector.tensor_copy(v4[t][:], ps[:])

            # ---- D1: q_lat^T (scaled) ----  (before qout opens; left stack)
            qlp = tc.alloc_tile_pool(name="qlatp", bufs=1)
            q_lat = [qlp.tile([P, S], BF, name=f"ql{m}", tag=f"ql{m}")
                     for m in range(QLT)]
            down_proj(tc, w_qd, QLT, q_lat, SCALE)

            with tc.tile_pool(name="qout", bufs=1, side="right") as qp:
                q_nope = [qp.tile([P, S], BF, name=f"qn{h}", tag=f"qn{h}")
                          for h in range(HPC)]
                q_rope = [qp.tile([P, S], BF, name=f"qr{p}", tag=f"qr{p}")
                          for p in range(HPC // 2)]

                # ---- U1: q-ups + rope-q ----
                with tc.tile_pool(name="wqup", bufs=1) as wp, \
                     tc.tile_pool(name="csp1", bufs=2) as csp, \
                     tc.tile_pool(name="tp1", bufs=1) as tp, \
                     tc.tile_pool(name="psu1", bufs=4, space="PSUM") as pu:
                    upp = [pu]
                    wu = [wp.tile([P, HPC * DN], BF, name=f"wqu{k}", tag=f"wqu{k}")
                          for k in range(QLT)]
                    wr = [wp.tile([P, HPC * DR], BF, name=f"wqr{k}", tag=f"wqr{k}")
                          for k in range(QLT)]
                    for k in range(QLT):
                        nc.sync.dma_start(out=wu[k][:], in_=w_qu[k * P:(k + 1) * P, :])
                        nc.sync.dma_start(out=wr[k][:], in_=w_qr[k * P:(k + 1) * P, :])
                    for n in range(NB):
                        for h in range(HPC):
                            up_proj(tc, pu, wu, QLT, q_lat, q_nope[h],
                                    slice(h * DN, (h + 1) * DN), n, h % 2)
                        for p in range(HPC // 2):
                            up_proj(tc, pu, wr, QLT, q_lat, q_rope[p],
                                    slice(p * 2 * DR, (p + 1) * 2 * DR), n, p % 2)
                            rope_block(tc, q_rope[p], csp, tp, n)
                qlp.release()

                # ---- ATT + o_proj per q-block ----
                with tc.tile_pool(name="att_c", bufs=1) as cp, \
                     tc.tile_pool(name="probs", bufs=4) as prp, \
                     tc.tile_pool(name="prftmp", bufs=2) as prf, \
                     tc.tile_pool(name="attn", bufs=5) as atp, \
                     tc.tile_pool(name="osb", bufs=2) as osp, \
                     tc.tile_pool(name="rdn", bufs=2) as rdp, \
                     tc.tile_pool(name="rbp", bufs=2) as rbp, \
                     tc.tile_pool(name="ps_s", bufs=2, space="PSUM") as ps_s, \
                     tc.tile_pool(name="ps_pv", bufs=2, space="PSUM") as ps_pv, \
                     tc.tile_pool(name="ps_den", bufs=1, space="PSUM") as ps_den, \
                     tc.tile_pool(name="ps_bc", bufs=1, space="PSUM") as ps_bc, \
                     tc.tile_pool(name="ps_o", bufs=2, space="PSUM") as ps_o:
                    wo = [cp.tile([P, HID], BF, name=f"wo{k}", tag=f"wo{k}")
                          for k in range(HPC)]
                    for k in range(HPC):
                        nc.sync.dma_start(out=wo[k][:], in_=w_o[k * P:(k + 1) * P, :])
                    ones_k = cp.tile([P, 1], BF, name="ones_k", tag="ones_k")
                    nc.vector.memset(ones_k[:], 1.0)
                    ones_m = cp.tile([1, P], BF, name="ones_m", tag="ones_m")
                    nc.vector.memset(ones_m[:], 1.0)
                    if variant == "causal":
                        msk = cp.tile([P, 896], FP, name="msk", tag="msk")
                        nc.gpsimd.memset(msk[:], 0.0)
                        nc.gpsimd.affine_select(
                            out=msk[:], in_=msk[:],
                            compare_op=mybir.AluOpType.is_ge,
                            fill=NEG, base=-384,
                            pattern=[[1, 896]], channel_multiplier=-1)
                    if variant == "generic":
                        mrp = tc.alloc_tile_pool(name="mrhs", bufs=18)

                    for j in range(NB):
                        nkt = 4 * (j + 1) if variant == "causal" else KT
                        mts = []
                        if variant == "generic":
                            for ki in range(KT):
                                mt = mrp.tile([P, 512], FP, name="mrhs", tag="mrhs")
                                nc.sync.dma_start(
                                    out=mt[:],
                                    in_=maskT[ki * P:(ki + 1) * P,
                                              j * 512:(j + 1) * 512])
                                mts.append(mt)
                        attn_sb = []
                        for h in range(HPC):
                            pp_, hh = h // 2, (h % 2) * DR
                            pv = ps_pv.tile([P, 512], FP)
                            den = ps_den.tile([1, 512], FP)
                            for ki in range(nkt):
                                ss = ps_s.tile([P, 512], FP)
                                nc.tensor.matmul(
                                    ss[:], k_nope[h][:, ki * P:(ki + 1) * P],
                                    q_nope[h][:, j * 512:(j + 1) * 512],
                                    start=True, stop=False)
                                nc.tensor.matmul(
                                    ss[:],
                                    k_rope[pp_][hh:hh + DR, ki * P:(ki + 1) * P],
                                    q_rope[pp_][hh:hh + DR, j * 512:(j + 1) * 512],
                                    start=False, stop=True)
                                pr = prp.tile([P, 512], BF, name="pr", tag="pr")
                                off = 128 * ki - 512 * j
                                if variant == "causal" and off >= 0:
                                    pf = prf.tile([P, 512], FP, name="pf", tag="pf")
                                    c0 = 384 - off
                                    nc.vector.tensor_tensor(
                                        pf[:], ss[:], msk[:, c0:c0 + 512],
                                        mybir.AluOpType.add)
                                    nc.scalar.activation(
                                        pr[:], pf[:],
                                        mybir.ActivationFunctionType.Exp)
                                elif variant == "generic":
                                    pf = prf.tile([P, 512], FP, name="pf", tag="pf")
                                    nc.vector.tensor_tensor(
                                        pf[:], ss[:], mts[ki][:],
                                        mybir.AluOpType.add)
                                    nc.scalar.activation(
                                        pr[:], pf[:],
                                        mybir.ActivationFunctionType.Exp)
                                else:
                                    nc.scalar.activation(
                                        pr[:], ss[:],
                                        mybir.ActivationFunctionType.Exp)
                                nc.tensor.matmul(pv[:],
                                                 v4[ki][:, h * DV:(h + 1) * DV],
                                                 pr[:], start=(ki == 0),
                                                 stop=(ki == nkt - 1))
                                nc.tensor.matmul(den[:], ones_k[:], pr[:],
                                                 start=(ki == 0),
                                                 stop=(ki == nkt - 1))
                            rden = rdp.tile([1, 512], BF, name="rden", tag="rden")
                            nc.vector.reciprocal(rden[:], den[:])
                            bc = ps_bc.tile([P, 512], FP)
                            nc.tensor.matmul(bc[:], ones_m[:], rden[:],
                                             start=True, stop=True)
                            rb = rbp.tile([P, 512], FP, name="rb", tag="rb")
                            nc.scalar.copy(rb[:], bc[:])
                            at = atp.tile([P, 512], BF, name="at", tag="at")
                            nc.vector.tensor_tensor(at[:], pv[:], rb[:],
                                                    mybir.AluOpType.mult)
                            attn_sb.append(at)
                        for t in range(4):
                            ob = osp.tile([P, HID], FP, name="ob", tag="ob")
                            for nn in range(NB):
                                po = ps_o.tile([P, 512], FP)
                                for kk in range(HPC):
                                    nc.tensor.matmul(
                                        po[:], attn_sb[kk][:, t * P:(t + 1) * P],
                                        wo[kk][:, nn * 512:(nn + 1) * 512],
                                        start=(kk == 0), stop=(kk == HPC - 1))
                                if nn % 2 == 0:
                                    nc.scalar.copy(ob[:, nn * 512:(nn + 1) * 512],
                                                   po[:])
                                else:
                                    nc.vector.tensor_copy(
                                        ob[:, nn * 512:(nn + 1) * 512], po[:])
                            nc.sync.dma_start(
                                out=o_out[(j * 4 + t) * P:(j * 4 + t + 1) * P, :],
                                in_=ob[:])
                    if variant == "generic":
                        mrp.release()

    nc.compile()
    return nc


def _get(variant):
    if variant not in _cache:
        _cache[variant] = _build(variant)
    return _cache[variant]


def _host_prep(inputs):
    hs = np.ascontiguousarray(inputs["hidden_states"], dtype=np.float32)
    mask = np.asarray(inputs["attention_mask"], dtype=np.float32)
    pos = np.asarray(inputs["position_ids"])
    B = hs.shape[0]

    causal = np.where(np.tril(np.ones((S, S), dtype=bool)), np.float32(0.0),
                      np.float32(NEG))
    variant = "causal"
    for b in range(B):
        if not np.array_equal(mask[b, 0], causal):
            variant = "zeros" if not mask.any() else "generic"
            break

    inv_freq = (1.0 / (ROPE_BASE ** (np.arange(0, DR, 2, dtype=np.float32) / DR)))
    css = []
    for b in range(B):
        t = pos[b].astype(np.float32)
        freqs = t[:, None] * inv_freq[None, :]  # [S, 32]
        cf = np.cos(freqs).T  # [32, S]
        sf = np.sin(freqs).T
        cs = np.empty((128, 2 * S), dtype=np.float32)
        for q in range(4):
            cs[q * 32:(q + 1) * 32, :S] = cf
            cs[q * 32:(q + 1) * 32, S:] = sf if q % 2 else -sf
        css.append(np.ascontiguousarray(cs.astype(NPBF)))
    return hs, mask, css, variant


def _make_in_maps(inputs, hs, mask, css, variant):
    def bf(x):
        return np.ascontiguousarray(np.asarray(x, dtype=np.float32).astype(NPBF))

    w_qd = bf(inputs["W_q_down"])
    w_kvd = bf(inputs["W_kv_down"])
    W_qu = np.asarray(inputs["W_q_up"], dtype=np.float32)
    W_qr = np.asarray(inputs["W_q_rope"], dtype=np.float32)
    W_ku = np.asarray(inputs["W_k_up"], dtype=np.float32)
    W_kr = np.asarray(inputs["W_k_rope"], dtype=np.float32)
    W_vu = np.asarray(inputs["W_v_up"], dtype=np.float32)
    W_o = np.asarray(inputs["W_o"], dtype=np.float32)

    hidT = [np.ascontiguousarray(hs[b].T.astype(NPBF)) for b in range(2)]
    maskT = [np.ascontiguousarray(mask[b, 0].T) for b in range(2)] \
        if variant == "generic" else None

    in_maps = []
    for core in range(NCORES):
        b, hg = divmod(core, NCORES // 2)
        m = {
            "hidT": hidT[b],
            "w_qd": w_qd,
            "w_kvd": w_kvd,
            "w_qu": bf(W_qu[:, hg * HPC * DN:(hg + 1) * HPC * DN]),
            "w_qr": bf(W_qr[:, hg * HPC * DR:(hg + 1) * HPC * DR]),
            "w_ku": bf(W_ku[:, hg * HPC * DN:(hg + 1) * HPC * DN]),
            "w_kr": bf(W_kr[:, hg * HPC * DR:(hg + 1) * HPC * DR]),
            "w_vu": bf(W_vu[:, hg * HPC * DV:(hg + 1) * HPC * DV]),
            "w_o": bf(W_o[hg * HPC * DV:(hg + 1) * HPC * DV, :]),
            "cs": css[b],
        }
        if maskT is not None:
            m["maskT"] = maskT[b]
        in_maps.append(m)
    return in_maps


_last_in_maps = None


def kernel(**inputs):
    global _last_in_maps
    hs, mask, css, variant = _host_prep(inputs)
    nc = _get(variant)
    in_maps = _make_in_maps(inputs, hs, mask, css, variant)
    _last_in_maps = in_maps

    res = run_bass_kernel_spmd(nc, in_maps, core_ids=list(range(NCORES)))
    out = np.zeros((2, S, HID), dtype=np.float32)
    for core in range(NCORES):
        b = core // (NCORES // 2)
        out[b] += res.results[core]["o"]
    return out


# revision 6
# speedup vs baseline: 2.8243x; 1.0677x over previous
"""MLA attention kernel for Trainium2, 8 NeuronCores.

Sharding: core = (batch b in {0,1}) x (group-rank g in {0..3}).
Role of g: token-block owner for the down-projections AND head-group owner
(4 heads) for ups/attention/o_proj.

Down-projections are deduplicated across the 4-core batch group: each core
computes q_lat^T/c_kv^T only for ITS 512-token block (from a host-sliced
hidT_my [HID,512]) and the full latents are assembled with two DRAM
AllGathers over replica groups [[0..3],[4..7]] (gather block r = group rank
r = token column block r on every core, so the program stays SPMD-static).

All matmul operands are bf16 (1 PE cycle/row vs 4 for fp32); accumulation
stays fp32 in PSUM, and attention scores stay fp32 through mask-add + exp
(only probs are rounded to bf16). Host casts inputs to bf16; o_proj output
partials are written bf16 and summed fp32 on host.

Layouts (transpose-free matmuls throughout):
  q_lat^T [768,S] -> q_nope^T/q_rope^T [d,S] (d-major)   (scores lhsT/rhs)
  c_kv^T [512,S]  -> k_nope^T/k_rope^T [d,S], V [S,dv]   (token-major V = PV lhsT)
  scores^T [k,q] -> exp -> probs^T (PV rhs), col-sum via ones-matmul,
  out^T [dv,q] normalized on PSUM-evict -> o_proj lhsT.
Rope dims are stored pair-packed: two heads per [128,S] tile (head A rows
0:64, head B rows 64:128); matmul APs slice at partition base 0/64.
Softmax skips fully-masked k/q tile pairs (causal) and uses no max-subtraction
(scores*scale is ~N(0,1); |s|<~8 so exp is safe in f32).
"""
import sys

sys.path.insert(0, "/opt/trn_rl_repo")

import numpy as np
import ml_dtypes
import concourse.bass as bass
import concourse.bacc as bacc
import concourse.tile as tile
from concourse import mybir
from concourse.bass_utils import run_bass_kernel_spmd

FP = mybir.dt.float32
BF = mybir.dt.bfloat16
NPBF = ml_dtypes.bfloat16
S = 2048
HID = 2048
H = 16
DN = 128
DR = 64
DV = 128
QL = 768
KVL = 512
ROPE_BASE = 10000.0
SCALE = (DN + DR) ** -0.5
NEG = -1e9
NCORES = 8
GSZ = NCORES // 2  # 4 cores per batch group
HPC = 4  # heads per core
P = 128
NB = S // 512  # 4 query/key column blocks of 512
KT = S // P  # 16 token tiles of 128
QLT = QL // P  # 6
KVT = KVL // P  # 4
REPLICA_GROUPS = [[0, 1, 2, 3], [4, 5, 6, 7]]

_cache = {}


def _build(variant):
    """variant: 'causal' (on-chip mask + tile skipping), 'zeros' (no mask),
    'generic' (mask^T DMA'd from DRAM, all tiles)."""
    nc = bacc.Bacc(num_devices=NCORES)

    hidT = nc.dram_tensor("hidT", [HID, 512], BF, kind="ExternalInput")
    w_qd = nc.dram_tensor("w_qd", [HID, QL], BF, kind="ExternalInput")
    w_kvd = nc.dram_tensor("w_kvd", [HID, KVL], BF, kind="ExternalInput")
    w_qu = nc.dram_tensor("w_qu", [QL, HPC * DN], BF, kind="ExternalInput")
    w_qr = nc.dram_tensor("w_qr", [QL, HPC * DR], BF, kind="ExternalInput")
    w_ku = nc.dram_tensor("w_ku", [KVL, HPC * DN], BF, kind="ExternalInput")
    w_kr = nc.dram_tensor("w_kr", [KVL, HPC * DR], BF, kind="ExternalInput")
    w_vu = nc.dram_tensor("w_vu", [KVL, HPC * DV], BF, kind="ExternalInput")
    w_o = nc.dram_tensor("w_o", [HPC * DV, HID], BF, kind="ExternalInput")
    cs = nc.dram_tensor("cs", [128, 2 * S], BF, kind="ExternalInput")
    if variant == "generic":
        maskT = nc.dram_tensor("maskT", [S, S], FP, kind="ExternalInput")
    o_out = nc.dram_tensor("o", [S, HID], BF, kind="ExternalOutput")

    def down_proj_slice(tc, w_dram, n_m, out_tiles, scale):
        # out_tiles: n_m SBUF tiles [128,512] = w^T @ hidT_my (my token block)
        with tc.tile_pool(name="wdp", bufs=1) as wp, \
             tc.tile_pool(name="rhs", bufs=4) as rp, \
             tc.tile_pool(name="psd", bufs=1, space="PSUM") as pp:
            wt = [wp.tile([P, n_m * P], BF, name=f"wd{k}", tag=f"wd{k}")
                  for k in range(KT)]
            for k in range(KT):
                nc.sync.dma_start(out=wt[k][:], in_=w_dram[k * P:(k + 1) * P, :])
            pss = [pp.tile([P, 512], FP, name=f"psd{m}", tag=f"psd{m}")
                   for m in range(n_m)]
            for k in range(KT):
                r = rp.tile([P, 512], BF, name="rhs", tag="rhs")
                nc.sync.dma_start(out=r[:], in_=hidT[k * P:(k + 1) * P, :])
                for m in range(n_m):
                    nc.tensor.matmul(pss[m][:], wt[k][:, m * P:(m + 1) * P], r[:],
                                     start=(k == 0), stop=(k == KT - 1))
            for m in range(n_m):
                if scale is not None:
                    nc.scalar.activation(out_tiles[m][:], pss[m][:],
                                         mybir.ActivationFunctionType.Copy,
                                         scale=float(scale))
                elif m % 2 == 0:
                    nc.scalar.copy(out_tiles[m][:], pss[m][:])
                else:
                    nc.vector.tensor_copy(out_tiles[m][:], pss[m][:])

    def rope_block(tc, x, cs_sb, tp, n):
        # in-place rope on x[:, n*512:(n+1)*512] of a pair-packed [128,S] tile
        cob = cs_sb[:, n * 512:(n + 1) * 512]
        snb = cs_sb[:, S + n * 512:S + (n + 1) * 512]
        xb = x[:, n * 512:(n + 1) * 512]
        t2 = tp.tile([P, 512], BF, name="t2", tag="t2")
        for q in range(4):
            src = (q // 2) * 64 + (32 if q % 2 == 0 else 0)
            # partition-shifted copy (1-input; shifted 2-input TT is rejected
            # by the compiler: base partitions must match)
            nc.vector.tensor_copy(t2[q * 32:(q + 1) * 32], xb[src:src + 32])
        nc.vector.tensor_tensor(t2[:], t2[:], snb, mybir.AluOpType.mult)
        nc.vector.tensor_tensor(xb, xb, cob, mybir.AluOpType.mult)
        nc.vector.tensor_tensor(xb, xb, t2[:], mybir.AluOpType.add)

    def up_proj(tc, pu, w_sb, kt, rhs_tiles, out_tile, h_cols, n, ev):
        # out_tile[:, nb] = w_sb[:, h_cols]^T @ rhs, contraction kt tiles
        ps = pu.tile([P, 512], FP, name="psu", tag="psu")
        for k in range(kt):
            nc.tensor.matmul(ps[:], w_sb[k][:, h_cols],
                             rhs_tiles[k][:, n * 512:(n + 1) * 512],
                             start=(k == 0), stop=(k == kt - 1))
        if ev == 0:
            nc.scalar.copy(out_tile[:, n * 512:(n + 1) * 512], ps[:])
        else:
            nc.vector.tensor_copy(out_tile[:, n * 512:(n + 1) * 512], ps[:])

    with nc.allow_low_precision(reason="bf16 kernel; 2e-2 rel-err budget"), \
         tile.TileContext(nc) as tc:
        with tc.tile_pool(name="kvout", bufs=1, side="right") as kvp, \
             tc.tile_pool(name="dramp", bufs=1, space="DRAM") as dramp:
            k_nope = [kvp.tile([P, S], BF, name=f"kn{h}", tag=f"kn{h}")
                      for h in range(HPC)]
            k_rope = [kvp.tile([P, S], BF, name=f"kr{p}", tag=f"kr{p}")
                      for p in range(HPC // 2)]
            v4 = [kvp.tile([P, HPC * DV], BF, name=f"v{t}", tag=f"v{t}")
                  for t in range(KT)]
            cs_sb = kvp.tile([P, 2 * S], BF, name="cs_sb", tag="cs_sb")
            nc.sync.dma_start(out=cs_sb[:], in_=cs[:, :])

            cin = dramp.tile([KVL, 512], BF, name="cin", tag="cin")
            cout = dramp.tile([GSZ * KVL, 512], BF, name="cout", tag="cout")
            qin = dramp.tile([QL, 512], BF, name="qin", tag="qin")
            qout = dramp.tile([GSZ * QL, 512], BF, name="qout_d", tag="qout_d")

            # ---- D2 slice: my c_kv^T block -> AllGather ----
            with tc.tile_pool(name="ckvs", bufs=1) as csl:
                ckv_s = [csl.tile([P, 512], BF, name=f"cks{m}", tag=f"cks{m}")
                         for m in range(KVT)]
                down_proj_slice(tc, w_kvd, KVT, ckv_s, None)
                for m in range(KVT):
                    nc.sync.dma_start(out=cin[m * P:(m + 1) * P, :],
                                      in_=ckv_s[m][:])
                nc.gpsimd.collective_compute(
                    "AllGather", mybir.AluOpType.bypass,
                    replica_groups=REPLICA_GROUPS,
                    ins=[cin.opt()], outs=[cout.opt()])

            # ---- D1 slice: my q_lat^T block (scaled) -> AllGather ----
            with tc.tile_pool(name="qls", bufs=1) as qsl:
                ql_s = [qsl.tile([P, 512], BF, name=f"qls{m}", tag=f"qls{m}")
                        for m in range(QLT)]
                down_proj_slice(tc, w_qd, QLT, ql_s, SCALE)
                for m in range(QLT):
                    nc.sync.dma_start(out=qin[m * P:(m + 1) * P, :],
                                      in_=ql_s[m][:])
                nc.gpsimd.collective_compute(
                    "AllGather", mybir.AluOpType.bypass,
                    replica_groups=REPLICA_GROUPS,
                    ins=[qin.opt()], outs=[qout.opt()])

            # ---- U2: k-ups + rope-k + V4 (full c_kv from gather) ----
            with tc.tile_pool(name="ckvp", bufs=1) as cp_:
                c_kv = [cp_.tile([P, S], BF, name=f"ckv{m}", tag=f"ckv{m}")
                        for m in range(KVT)]
                for m in range(KVT):
                    for r in range(GSZ):
                        nc.sync.dma_start(
                            out=c_kv[m][:, r * 512:(r + 1) * 512],
                            in_=cout[r * KVL + m * P:r * KVL + (m + 1) * P, :])

                with tc.tile_pool(name="wku", bufs=1) as wp, \
                     tc.tile_pool(name="tp2", bufs=2) as tp, \
                     tc.tile_pool(name="psu2", bufs=3, space="PSUM") as pu:
                    ku = [wp.tile([P, HPC * DN], BF, name=f"wku{k}", tag=f"wku{k}")
                          for k in range(KVT)]
                    kr = [wp.tile([P, HPC * DR], BF, name=f"wkr{k}", tag=f"wkr{k}")
                          for k in range(KVT)]
                    vu = [wp.tile([P, HPC * DV], BF, name=f"wvu{k}", tag=f"wvu{k}")
                          for k in range(KVT)]
                    for k in range(KVT):
                        nc.sync.dma_start(out=ku[k][:], in_=w_ku[k * P:(k + 1) * P, :])
                        nc.sync.dma_start(out=kr[k][:], in_=w_kr[k * P:(k + 1) * P, :])
                        nc.sync.dma_start(out=vu[k][:], in_=w_vu[k * P:(k + 1) * P, :])
                    for n in range(NB):
                        for h in range(HPC):
                            up_proj(tc, pu, ku, KVT, c_kv, k_nope[h],
                                    slice(h * DN, (h + 1) * DN), n, h % 2)
                        for p in range(HPC // 2):
                            up_proj(tc, pu, kr, KVT, c_kv, k_rope[p],
                                    slice(p * 2 * DR, (p + 1) * 2 * DR), n, p % 2)
                            rope_block(tc, k_rope[p], cs_sb, tp, n)
                    for t in range(KT):
                        ps = pu.tile([P, HPC * DV], FP, name="psv", tag="psv")
                        for k in range(KVT):
                            nc.tensor.matmul(ps[:], c_kv[k][:, t * P:(t + 1) * P],
                                             vu[k][:], start=(k == 0),
                                             stop=(k == KVT - 1))
                        if t % 2 == 0:
                            nc.scalar.copy(v4[t][:], ps[:])
                        else:
                            nc.vector.tensor_copy(v4[t][:], ps[:])

            with tc.tile_pool(name="qout", bufs=1, side="right") as qp:
                q_nope = [qp.tile([P, S], BF, name=f"qn{h}", tag=f"qn{h}")
                          for h in range(HPC)]
                q_rope = [qp.tile([P, S], BF, name=f"qr{p}", tag=f"qr{p}")
                          for p in range(HPC // 2)]

                # ---- U1: q-ups + rope-q (full q_lat from gather) ----
                with tc.tile_pool(name="qlatp", bufs=1) as qlp:
                    q_lat = [qlp.tile([P, S], BF, name=f"ql{m}", tag=f"ql{m}")
                             for m in range(QLT)]
                    for m in range(QLT):
                        for r in range(GSZ):
                            nc.sync.dma_start(
                                out=q_lat[m][:, r * 512:(r + 1) * 512],
                                in_=qout[r * QL + m * P:r * QL + (m + 1) * P, :])
                    with tc.tile_pool(name="wqup", bufs=1) as wp, \
                         tc.tile_pool(name="tp1", bufs=1) as tp, \
                         tc.tile_pool(name="psu1", bufs=4, space="PSUM") as pu:
                        wu = [wp.tile([P, HPC * DN], BF, name=f"wqu{k}",
                                      tag=f"wqu{k}") for k in range(QLT)]
                        wr = [wp.tile([P, HPC * DR], BF, name=f"wqr{k}",
                                      tag=f"wqr{k}") for k in range(QLT)]
                        for k in range(QLT):
                            nc.sync.dma_start(out=wu[k][:],
                                              in_=w_qu[k * P:(k + 1) * P, :])
                            nc.sync.dma_start(out=wr[k][:],
                                              in_=w_qr[k * P:(k + 1) * P, :])
                        for n in range(NB):
                            for h in range(HPC):
                                up_proj(tc, pu, wu, QLT, q_lat, q_nope[h],
                                        slice(h * DN, (h + 1) * DN), n, h % 2)
                            for p in range(HPC // 2):
                                up_proj(tc, pu, wr, QLT, q_lat, q_rope[p],
                                        slice(p * 2 * DR, (p + 1) * 2 * DR), n,
                                        p % 2)
                                rope_block(tc, q_rope[p], cs_sb, tp, n)

                # ---- ATT + o_proj per q-block ----
                with tc.tile_pool(name="att_c", bufs=1) as cp, \
                     tc.tile_pool(name="probs", bufs=4) as prp, \
                     tc.tile_pool(name="prftmp", bufs=2) as prf, \
                     tc.tile_pool(name="attn", bufs=5) as atp, \
                     tc.tile_pool(name="osb", bufs=2) as osp, \
                     tc.tile_pool(name="rdn", bufs=2) as rdp, \
                     tc.tile_pool(name="rbp", bufs=2) as rbp, \
                     tc.tile_pool(name="ps_s", bufs=2, space="PSUM") as ps_s, \
                     tc.tile_pool(name="ps_pv", bufs=2, space="PSUM") as ps_pv, \
                     tc.tile_pool(name="ps_den", bufs=1, space="PSUM") as ps_den, \
                     tc.tile_pool(name="ps_bc", bufs=1, space="PSUM") as ps_bc, \
                     tc.tile_pool(name="ps_o", bufs=2, space="PSUM") as ps_o:
                    wo = [cp.tile([P, HID], BF, name=f"wo{k}", tag=f"wo{k}")
                          for k in range(HPC)]
                    for k in range(HPC):
                        nc.sync.dma_start(out=wo[k][:], in_=w_o[k * P:(k + 1) * P, :])
                    ones_k = cp.tile([P, 1], BF, name="ones_k", tag="ones_k")
                    nc.vector.memset(ones_k[:], 1.0)
                    ones_m = cp.tile([1, P], BF, name="ones_m", tag="ones_m")
                    nc.vector.memset(ones_m[:], 1.0)
                    if variant == "causal":
                        msk = cp.tile([P, 896], FP, name="msk", tag="msk")
                        nc.gpsimd.memset(msk[:], 0.0)
                        nc.gpsimd.affine_select(
                            out=msk[:], in_=msk[:],
                            compare_op=mybir.AluOpType.is_ge,
                            fill=NEG, base=-384,
                            pattern=[[1, 896]], channel_multiplier=-1)
                    if variant == "generic":
                        mrp = tc.alloc_tile_pool(name="mrhs", bufs=18)

                    for j in range(NB):
                        nkt = 4 * (j + 1) if variant == "causal" else KT
                        mts = []
                        if variant == "generic":
                            for ki in range(KT):
                                mt = mrp.tile([P, 512], FP, name="mrhs", tag="mrhs")
                                nc.sync.dma_start(
                                    out=mt[:],
                                    in_=maskT[ki * P:(ki + 1) * P,
                                              j * 512:(j + 1) * 512])
                                mts.append(mt)
                        attn_sb = []
                        for h in range(HPC):
                            pp_, hh = h // 2, (h % 2) * DR
                            pv = ps_pv.tile([P, 512], FP)
                            den = ps_den.tile([1, 512], FP)
                            for ki in range(nkt):
                                ss = ps_s.tile([P, 512], FP)
                                nc.tensor.matmul(
                                    ss[:], k_nope[h][:, ki * P:(ki + 1) * P],
                                    q_nope[h][:, j * 512:(j + 1) * 512],
                                    start=True, stop=False)
                                nc.tensor.matmul(
                                    ss[:],
                                    k_rope[pp_][hh:hh + DR, ki * P:(ki + 1) * P],
                                    q_rope[pp_][hh:hh + DR, j * 512:(j + 1) * 512],
                                    start=False, stop=True)
                                pr = prp.tile([P, 512], BF, name="pr", tag="pr")
                                off = 128 * ki - 512 * j
                                if variant == "causal" and off >= 0:
                                    pf = prf.tile([P, 512], FP, name="pf", tag="pf")
                                    c0 = 384 - off
                                    nc.vector.tensor_tensor(
                                        pf[:], ss[:], msk[:, c0:c0 + 512],
                                        mybir.AluOpType.add)
                                    nc.scalar.activation(
                                        pr[:], pf[:],
                                        mybir.ActivationFunctionType.Exp)
                                elif variant == "generic":
                                    pf = prf.tile([P, 512], FP, name="pf", tag="pf")
                                    nc.vector.tensor_tensor(
                                        pf[:], ss[:], mts[ki][:],
                                        mybir.AluOpType.add)
                                    nc.scalar.activation(
                                        pr[:], pf[:],
                                        mybir.ActivationFunctionType.Exp)
                                else:
                                    nc.scalar.activation(
                                        pr[:], ss[:],
                                        mybir.ActivationFunctionType.Exp)
                                nc.tensor.matmul(pv[:],
                                                 v4[ki][:, h * DV:(h + 1) * DV],
                                                 pr[:], start=(ki == 0),
                                                 stop=(ki == nkt - 1))
                                nc.tensor.matmul(den[:], ones_k[:], pr[:],
                                                 start=(ki == 0),
                                                 stop=(ki == nkt - 1))
                            rden = rdp.tile([1, 512], BF, name="rden", tag="rden")
                            nc.vector.reciprocal(rden[:], den[:])
                            bc = ps_bc.tile([P, 512], FP)
                            nc.tensor.matmul(bc[:], ones_m[:], rden[:],
                                             start=True, stop=True)
                            rb = rbp.tile([P, 512], FP, name="rb", tag="rb")
                            nc.scalar.copy(rb[:], bc[:])
                            at = atp.tile([P, 512], BF, name="at", tag="at")
                            nc.vector.tensor_tensor(at[:], pv[:], rb[:],
                                                    mybir.AluOpType.mult)
                            attn_sb.append(at)
                        for t in range(4):
                            ob = osp.tile([P, HID], BF, name="ob", tag="ob")
                            for nn in range(NB):
                                po = ps_o.tile([P, 512], FP)
                                for kk in range(HPC):
                                    nc.tensor.matmul(
                                        po[:], attn_sb[kk][:, t * P:(t + 1) * P],
                                        wo[kk][:, nn * 512:(nn + 1) * 512],
                                        start=(kk == 0), stop=(kk == HPC - 1))
                                if nn % 2 == 0:
                                    nc.scalar.copy(ob[:, nn * 512:(nn + 1) * 512],
                                                   po[:])
                                else:
                                    nc.vector.tensor_copy(
                                        ob[:, nn * 512:(nn + 1) * 512], po[:])
                            nc.sync.dma_start(
                                out=o_out[(j * 4 + t) * P:(j * 4 + t + 1) * P, :],
                                in_=ob[:])
                    if variant == "generic":
                        mrp.release()

    nc.compile()
    return nc


def _get(variant):
    if variant not in _cache:
        _cache[variant] = _build(variant)
    return _cache[variant]


def _host_prep(inputs):
    hs = np.ascontiguousarray(inputs["hidden_states"], dtype=np.float32)
    mask = np.asarray(inputs["attention_mask"], dtype=np.float32)
    pos = np.asarray(inputs["position_ids"])
    B = hs.shape[0]

    causal = np.where(np.tril(np.ones((S, S), dtype=bool)), np.float32(0.0),
                      np.float32(NEG))
    variant = "causal"
    for b in range(B):
        if not np.array_equal(mask[b, 0], causal):
            variant = "zeros" if not mask.any() else "generic"
            break

    inv_freq = (1.0 / (ROPE_BASE ** (np.arange(0, DR, 2, dtype=np.float32) / DR)))
    css = []
    for b in range(B):
        t = pos[b].astype(np.float32)
        freqs = t[:, None] * inv_freq[None, :]  # [S, 32]
        cf = np.cos(freqs).T  # [32, S]
        sf = np.sin(freqs).T
        cs = np.empty((128, 2 * S), dtype=np.float32)
        for q in range(4):
            cs[q * 32:(q + 1) * 32, :S] = cf
            cs[q * 32:(q + 1) * 32, S:] = sf if q % 2 else -sf
        css.append(np.ascontiguousarray(cs.astype(NPBF)))
    return hs, mask, css, variant


def _make_in_maps(inputs, hs, mask, css, variant):
    def bf(x):
        return np.ascontiguousarray(np.asarray(x, dtype=np.float32).astype(NPBF))

    w_qd = bf(inputs["W_q_down"])
    w_kvd = bf(inputs["W_kv_down"])
    W_qu = np.asarray(inputs["W_q_up"], dtype=np.float32)
    W_qr = np.asarray(inputs["W_q_rope"], dtype=np.float32)
    W_ku = np.asarray(inputs["W_k_up"], dtype=np.float32)
    W_kr = np.asarray(inputs["W_k_rope"], dtype=np.float32)
    W_vu = np.asarray(inputs["W_v_up"], dtype=np.float32)
    W_o = np.asarray(inputs["W_o"], dtype=np.float32)

    hidT = [np.asarray(hs[b].T.astype(NPBF)) for b in range(2)]
    maskT = [np.ascontiguousarray(mask[b, 0].T) for b in range(2)] \
        if variant == "generic" else None

    in_maps = []
    for core in range(NCORES):
        b, g = divmod(core, GSZ)
        m = {
            "hidT": np.ascontiguousarray(hidT[b][:, g * 512:(g + 1) * 512]),
            "w_qd": w_qd,
            "w_kvd": w_kvd,
            "w_qu": bf(W_qu[:, g * HPC * DN:(g + 1) * HPC * DN]),
            "w_qr": bf(W_qr[:, g * HPC * DR:(g + 1) * HPC * DR]),
            "w_ku": bf(W_ku[:, g * HPC * DN:(g + 1) * HPC * DN]),
            "w_kr": bf(W_kr[:, g * HPC * DR:(g + 1) * HPC * DR]),
            "w_vu": bf(W_vu[:, g * HPC * DV:(g + 1) * HPC * DV]),
            "w_o": bf(W_o[g * HPC * DV:(g + 1) * HPC * DV, :]),
            "cs": css[b],
        }
        if maskT is not None:
            m["maskT"] = maskT[b]
        in_maps.append(m)
    return in_maps


_last_in_maps = None


def kernel(**inputs):
    global _last_in_maps
    hs, mask, css, variant = _host_prep(inputs)
    nc = _get(variant)
    in_maps = _make_in_maps(inputs, hs, mask, css, variant)
    _last_in_maps = in_maps

    res = run_bass_kernel_spmd(nc, in_maps, core_ids=list(range(NCORES)))
    out = np.zeros((2, S, HID), dtype=np.float32)
    for core in range(NCORES):
        b = core // GSZ
        out[b] += res.results[core]["o"].astype(np.float32)
    return out
